# revision 7
# baseline (speedup 1.0000x reference)
"""Burger dissipative loss operator on 8 TRN2 NeuronCores — v2.

Math (reference):
    u   = x_t[:, 0];  u1 = x_t1[:, 0];  len = edge_attr[:, 0]
    temporal = (u - u1) / dt
    du  = scatter_mean over dst of (u1[dst] - u1[src]) / len
    d2u = scatter_mean over dst of (du[dst] - du[src]) / len
    loss = (temporal + du * u1 - mu * d2u) * mask

Per dst d (w = 1/len):
    du[d] = (u1[d] * A[d] - B[d]) / max(deg[d], 1)
    A[d]  = sum_e w[e],  B[d] = sum_e w[e]*u1[src[e]]

Layout: edges partitioned by dst range across 8 cores.  Within a core the
local dsts are sorted by degree (descending) and dealt round-robin across
the 128 partitions, so every row of 128 dsts shares one run width; the
segment sums then reduce with a handful of strided tensor_reduce calls
(one per distinct degree) — no scans, no boundary extraction.  The only
data-dependent device ops are one indirect-DMA gather of u1[src] (round 1)
/ du[src] (round 2) per edge, issued per column in chunked tiles.  du is
exchanged between rounds with an on-device AllGather.
"""

import os
import sys

for _p in ("/opt/trn_rl_repo", "/root/.axon_site/_ro/trn_rl_repo"):
    if os.path.isdir(_p) and _p not in sys.path:
        sys.path.insert(0, _p)

import numpy as np

import concourse.bass as bass
import concourse.mybir as mybir
import concourse.tile as tile
from concourse import bass_utils
from concourse.vector_clock import ScopedClock

F32 = mybir.dt.float32
I32 = mybir.dt.int32

_drain_patched = False


def _install_drain_patch():
    global _drain_patched
    if _drain_patched:
        return
    _drain_patched = True

    def _drain_and_barrier(self, tick_clock, wait_clock):
        nc = self.nc
        sink = nc.sync.nop(nofuse=True)
        wait_clock.add_sem_waits(
            sink.ins, ScopedClock({None: tick_clock.global_clock}))
        waits = list(sink.ins.sync_info.on_wait) if sink.ins.sync_info else []
        if len(waits) > 1:
            sink.ins.sync_info = mybir.SyncInfo(
                on_wait=waits[:1], on_update=list(sink.ins.sync_info.on_update))
            rest = waits[1:]
            while rest:
                extra = nc.sync.nop(nofuse=True)
                upd = (list(extra.ins.sync_info.on_update)
                       if extra.ins.sync_info else [])
                extra.ins.sync_info = mybir.SyncInfo(
                    on_wait=rest[:1], on_update=upd)
                rest = rest[1:]
        nc.sync.drain()
        nc.all_engine_barrier()
        assert self.sems is not None
        popped = nc._tile_sem_poison_stack.pop()
        assert popped is self._sem_poison
        nc.clear_and_free_semaphores(list(self.sems.allocated().values()))
        nc.all_engine_barrier()

    tile.TileContext._drain_and_barrier = _drain_and_barrier

    _orig_commit = tile.TileContext._commit_instruction
    _ctr = [0]

    def _commit_instruction(self, inst, lazy_reg_writes=True):
        si = getattr(inst, "sync_info", None)
        if (si is not None and si.on_wait and len(si.on_wait) > 1
                and inst.engine != mybir.EngineType.Unassigned):
            waits = list(si.on_wait)
            inst.sync_info = mybir.SyncInfo(
                on_wait=[waits[-1]], on_update=list(si.on_update))
            for w in waits[:-1]:
                _ctr[0] += 1
                nop = mybir.InstNoOp(name=f"I-ws{_ctr[0]}", ins=[], outs=[])
                nop.engine = inst.engine
                nop.sync_info = mybir.SyncInfo(on_wait=[w], on_update=[])
                self._add_instruction(nop)
        return _orig_commit(self, inst, lazy_reg_writes)

    tile.TileContext._commit_instruction = _commit_instruction


P = 128
NCORES = 8
DELTA_T = 0.01
MU = 0.01
NSWQ = 1


def _indirect_gather_q(nc, out, table_ap, off_ap, queue_num):
    """indirect_dma_start clone with SWDGE queue selection."""
    eng = nc.gpsimd
    out_ap = eng.lower_ap_dma(out, for_indirect_dma=True)
    in_ap = eng.lower_ap_dma(table_ap, for_indirect_dma=True)
    offset_lowered = eng.lower_ap_dma(off_ap)
    in_ap.append(offset_lowered[0])
    ap_shape = table_ap.shape
    coef = 1
    for i in range(1, len(ap_shape)):
        coef *= ap_shape[i]
    in_ap[0].dynamic_ap_info = mybir.DynamicAccessPatternInfo(
        c=0,
        actual_ap=out.ap,
        indirect_dim_max_index=ap_shape[0],
        offset_expr=[
            mybir.DynamicAccessPatternOffsetExpr(
                coef=coef,
                aff_expr=mybir.DynamicAccessPatternOffsetExprAffExpr(
                    kind="IndirectArgId", arg_id=1,
                ),
            )
        ],
    )
    return eng.add_instruction(
        mybir.InstDMACopy(
            name=eng.bass.get_next_instruction_name(),
            queue=f"qPoolDynamic{queue_num or ''}",
            mode="Copy",
            ins=in_ap,
            outs=out_ap,
            oob_is_err=True,
            cce_op=mybir.AluOpType.bypass,
        )
    )


# ---------------------------------------------------------------------------
# Host-side preprocessing
# ---------------------------------------------------------------------------

def _preprocess(x_t, x_t1, edge_index, edge_attr, mask):
    N = x_t.shape[0]
    E = edge_index.shape[1]
    NL = N // NCORES
    NR = -(-NL // P)          # dealt rows per core
    NRP = NR * P
    DUL = P * NR

    src = np.ascontiguousarray(edge_index[0]).astype(np.int64, copy=False)
    dst = np.ascontiguousarray(edge_index[1]).astype(np.int64, copy=False)
    w_all = (np.float32(1.0) / edge_attr[:, 0].astype(np.float32))

    order = np.argsort(dst, kind="stable")
    ds = dst[order]
    ss = src[order].astype(np.int64)
    ws = w_all[order]
    core_cuts = np.searchsorted(ds, np.arange(NCORES + 1) * NL)

    u_full = np.ascontiguousarray(x_t[:, 0]).astype(np.float32)
    u1_full = np.ascontiguousarray(x_t1[:, 0]).astype(np.float32)
    mask_full = np.ascontiguousarray(mask[:, 0]).astype(np.float32)

    cores = []
    Cmax = 0
    for k in range(NCORES):
        lo, hi = core_cuts[k], core_cuts[k + 1]
        dloc = ds[lo:hi] - k * NL
        deg = np.zeros(NRP, np.int64)
        deg[:NL] = np.bincount(dloc, minlength=NL)
        cumdeg = np.concatenate([[0], np.cumsum(deg[:NL])])
        # degree-descending deal
        D = np.argsort(-deg, kind="stable")          # dealt id -> local dst id
        rank = np.empty(NRP, np.int64)
        rank[D] = np.arange(NRP)
        rowdeg = deg[D[0::P]]                        # width of each row (max)
        off = np.concatenate([[0], np.cumsum(rowdeg)])
        C = int(off[-1])
        Cmax = max(Cmax, C)
        cores.append(dict(lo=lo, hi=hi, deg=deg, cumdeg=cumdeg, D=D,
                          rank=rank, rowdeg=rowdeg, off=off, C=C))

    C = -(-Cmax // 4) * 4

    # global du index of node n: core k, dealt (p, j) -> k*DUL + p*NR + j
    g_of_node = np.empty(N, np.int64)
    for k in range(NCORES):
        pc = cores[k]
        r = pc["rank"][:NL]
        g_of_node[k * NL:(k + 1) * NL] = k * DUL + (r % P) * NR + (r // P)

    in_maps = []
    for k in range(NCORES):
        pc = cores[k]
        lo, hi = pc["lo"], pc["hi"]
        dloc = ds[lo:hi] - k * NL
        src_k = ss[lo:hi]
        w_k = ws[lo:hi]
        # per-edge dealt placement
        r = pc["rank"][dloc]
        lane = r % P
        row = r // P
        within = np.arange(hi - lo) - pc["cumdeg"][dloc]
        col = pc["off"][row] + within

        src1 = np.zeros((P, C), np.int32)
        w_arr = np.zeros((P, C), np.float32)
        src2 = np.zeros((P, C), np.int32)
        src1[lane, col] = src_k
        w_arr[lane, col] = w_k
        src2[lane, col] = g_of_node[src_k]

        D = pc["D"]
        gids = np.where(D < NL, k * NL + D, 0)       # dummy -> node 0
        valid = (D < NL).astype(np.float32)
        u1_loc = (u1_full[gids] * valid).reshape(NR, P).T.copy()
        u_loc = (u_full[gids] * valid).reshape(NR, P).T.copy()
        m_loc = (mask_full[gids] * valid).reshape(NR, P).T.copy()
        inv_c = (1.0 / np.maximum(pc["deg"][D], 1)).astype(np.float32)
        inv_c = inv_c.reshape(NR, P).T.copy()

        in_maps.append(dict(
            table1=u1_full.reshape(N, 1),
            src1=src1, src2=src2, w=w_arr,
            u1_loc=np.ascontiguousarray(u1_loc),
            u_loc=np.ascontiguousarray(u_loc),
            m_loc=np.ascontiguousarray(m_loc),
            inv_c=np.ascontiguousarray(inv_c),
        ))

    # reduce sections shared across cores?  rowdeg differs per core -> per-core
    sections = []
    for k in range(NCORES):
        rowdeg = cores[k]["rowdeg"]
        sec = []
        j = 0
        while j < NR:
            wdt = int(rowdeg[j])
            j2 = j
            while j2 < NR and int(rowdeg[j2]) == wdt:
                j2 += 1
            if wdt > 0:
                sec.append((j, j2, wdt, int(cores[k]["off"][j])))
            j = j2
        sections.append(sec)

    dims = dict(N=N, E=E, NL=NL, NR=NR, C=C, DUL=DUL)
    meta = [dict(D=cores[k]["D"]) for k in range(NCORES)]
    return in_maps, meta, dims, sections


# ---------------------------------------------------------------------------
# Device kernel (per-core program; sections differ per core -> build per core?
# run_bass_kernel_spmd shares one program across cores, so sections must be
# identical.  Degrees are iid Poisson across cores, so the row-degree
# profiles are nearly but not exactly equal; use the per-core maximum width
# per row so one program covers all cores.
# ---------------------------------------------------------------------------

def _build_nc(dims, sections, ncores=NCORES):
    N, C, NR, DUL = dims["N"], dims["C"], dims["NR"], dims["DUL"]
    add = mybir.AluOpType.add
    sub = mybir.AluOpType.subtract
    mult = mybir.AluOpType.mult

    _install_drain_patch()
    nc = bass.Bass("TRN2", target_bir_lowering=False, debug=False,
                   num_devices=ncores, num_swdge_queues=NSWQ)

    table1 = nc.dram_tensor("table1", [N, 1], F32, kind="ExternalInput")
    src1_d = nc.dram_tensor("src1", [P, C], I32, kind="ExternalInput")
    src2_d = nc.dram_tensor("src2", [P, C], I32, kind="ExternalInput")
    w_d = nc.dram_tensor("w", [P, C], F32, kind="ExternalInput")
    u1_loc_d = nc.dram_tensor("u1_loc", [P, NR], F32, kind="ExternalInput")
    u_loc_d = nc.dram_tensor("u_loc", [P, NR], F32, kind="ExternalInput")
    m_loc_d = nc.dram_tensor("m_loc", [P, NR], F32, kind="ExternalInput")
    inv_c_d = nc.dram_tensor("inv_c", [P, NR], F32, kind="ExternalInput")
    loss_d = nc.dram_tensor("loss", [P, NR], F32, kind="ExternalOutput")

    du_slice = nc.dram_tensor("du_slice", [DUL], F32)
    du_full = nc.dram_tensor("du_full", [ncores * DUL, 1], F32)

    def reduces(out_t, in_t):
        for (j0, j1, wdt, c0) in sections:
            n = j1 - j0
            if wdt == 1:
                nc.vector.tensor_copy(out=out_t[:, j0:j1],
                                      in_=in_t[:, c0:c0 + n])
            else:
                nc.vector.tensor_reduce(
                    out=out_t[:, j0:j1],
                    in_=in_t[:, c0:c0 + n * wdt].rearrange(
                        "p (n w) -> p n w", w=wdt),
                    axis=mybir.AxisListType.X, op=add)

    CH = 1024
    n_ch = -(-C // CH)
    with tile.TileContext(nc) as tc:
        with tc.tile_pool(name="persist", bufs=1) as pp, \
             tc.tile_pool(name="stream", bufs=3) as sp:
            w_t = pp.tile([P, C], F32, tag="w")
            u1_loc_t = pp.tile([P, NR], F32, tag="u1")
            inv_c_t = pp.tile([P, NR], F32, tag="ic")

            g_t = pp.tile([P, C], F32, tag="g")
            A_t = pp.tile([P, NR], F32, tag="A")
            s_t = pp.tile([P, NR], F32, tag="s")
            du_t = pp.tile([P, NR], F32, tag="du")

            nc.vector.memset(A_t[:], 0.0)
            nc.vector.memset(s_t[:], 0.0)

            # ---- round 1 (chunked idx tiles + chunked gather dests) --------
            for j in range(n_ch):
                c0 = j * CH
                c1 = min(C, c0 + CH)
                idx_c = sp.tile([P, CH], I32, tag="idxc")
                nc.sync.dma_start(out=idx_c[:, :c1 - c0],
                                  in_=src1_d[:, c0:c1])
                if j == 0:
                    nc.sync.dma_start(out=w_t[:], in_=w_d[:])
                    nc.sync.dma_start(out=u1_loc_t[:], in_=u1_loc_d[:])
                    nc.sync.dma_start(out=inv_c_t[:], in_=inv_c_d[:])
                gc = sp.tile([P, CH], F32, tag="gc")
                for i in range(c1 - c0):
                    _indirect_gather_q(nc, gc[:, i:i + 1], table1[:],
                                       idx_c[:, i:i + 1], i % NSWQ)
                nc.vector.tensor_tensor(out=g_t[:, c0:c1],
                                        in0=gc[:, :c1 - c0],
                                        in1=w_t[:, c0:c1], op=mult)
            reduces(A_t, w_t)                      # A = seg-sum of w
            reduces(s_t, g_t)                      # B = seg-sum of w*u1[src]
            # du = (u1*A - B) * inv_c
            nc.vector.tensor_tensor(out=du_t[:], in0=u1_loc_t[:], in1=A_t[:],
                                    op=mult)
            nc.vector.tensor_tensor(out=du_t[:], in0=du_t[:], in1=s_t[:],
                                    op=sub)
            nc.vector.tensor_tensor(out=du_t[:], in0=du_t[:], in1=inv_c_t[:],
                                    op=mult)

            # ---- allgather du ----------------------------------------------
            nc.sync.dma_start(
                out=du_slice[:].rearrange("(p c) -> p c", p=P), in_=du_t[:])
            nc.gpsimd.collective_compute(
                "AllGather", mybir.AluOpType.bypass,
                replica_groups=[list(range(ncores))],
                ins=[du_slice.ap().opt()],
                outs=[du_full.ap().rearrange("n one -> (n one)").opt()])

            # ---- round 2 (chunked) -----------------------------------------
            for j in range(n_ch):
                c0 = j * CH
                c1 = min(C, c0 + CH)
                idx_c = sp.tile([P, CH], I32, tag="idxc")
                nc.sync.dma_start(out=idx_c[:, :c1 - c0],
                                  in_=src2_d[:, c0:c1])
                gc = sp.tile([P, CH], F32, tag="gc")
                for i in range(c1 - c0):
                    _indirect_gather_q(nc, gc[:, i:i + 1], du_full[:],
                                       idx_c[:, i:i + 1], i % NSWQ)
                nc.vector.tensor_tensor(out=g_t[:, c0:c1],
                                        in0=gc[:, :c1 - c0],
                                        in1=w_t[:, c0:c1], op=mult)
            # no memset: round-2 reduces overwrite every width>0 row, and
            # width-0 rows still hold 0 from the initial memset
            reduces(s_t, g_t)
            # d2u = (du*A - s2) * inv_c   -> into A_t
            nc.vector.tensor_tensor(out=A_t[:], in0=du_t[:], in1=A_t[:],
                                    op=mult)
            nc.vector.tensor_tensor(out=A_t[:], in0=A_t[:], in1=s_t[:],
                                    op=sub)
            nc.vector.tensor_tensor(out=A_t[:], in0=A_t[:], in1=inv_c_t[:],
                                    op=mult)

            # ---- final loss -------------------------------------------------
            u_loc_t = g_t[:, :NR]                    # g slot dead after s2
            nc.sync.dma_start(out=u_loc_t, in_=u_loc_d[:])
            m_loc_t = g_t[:, NR:2 * NR]
            nc.sync.dma_start(out=m_loc_t, in_=m_loc_d[:])
            # s_t := u - u1
            nc.vector.tensor_tensor(out=s_t[:], in0=u_loc_t, in1=u1_loc_t[:],
                                    op=sub)
            # du := du * u1
            nc.vector.tensor_tensor(out=du_t[:], in0=du_t[:], in1=u1_loc_t[:],
                                    op=mult)
            # s = s/dt + du*u1
            nc.vector.scalar_tensor_tensor(
                out=s_t[:], in0=s_t[:], scalar=1.0 / DELTA_T, in1=du_t[:],
                op0=mult, op1=add)
            # s = -mu*d2u + s
            nc.vector.scalar_tensor_tensor(
                out=s_t[:], in0=A_t[:], scalar=-MU, in1=s_t[:],
                op0=mult, op1=add)
            nc.vector.tensor_tensor(out=s_t[:], in0=s_t[:], in1=m_loc_t,
                                    op=mult)
            nc.sync.dma_start(out=loss_d[:], in_=s_t[:])

    return nc


# ---------------------------------------------------------------------------
# Entry point
# ---------------------------------------------------------------------------

def kernel(x_t, x_t1, edge_index, edge_attr, mask, _trace=False):
    x_t = np.asarray(x_t)
    x_t1 = np.asarray(x_t1)
    edge_index = np.asarray(edge_index)
    edge_attr = np.asarray(edge_attr)
    mask = np.asarray(mask)
    N = x_t.shape[0]
    NL = N // NCORES

    in_maps, meta, dims, sections = _preprocess(
        x_t, x_t1, edge_index, edge_attr, mask)

    # one program across cores: use the row-wise max width over cores
    NR = dims["NR"]
    roww = np.zeros(NR, np.int64)
    secw = [np.zeros(NR, np.int64) for _ in range(NCORES)]
    for k in range(NCORES):
        for (j0, j1, wdt, c0) in sections[k]:
            secw[k][j0:j1] = wdt
        roww = np.maximum(roww, secw[k])
    off = np.concatenate([[0], np.cumsum(roww)])
    C = -(-int(off[-1]) // 4) * 4
    dims["C"] = C
    # repack per-core edge arrays into the common layout
    for k in range(NCORES):
        sk = secw[k]
        src1 = np.zeros((P, C), np.int32)
        src2 = np.zeros((P, C), np.int32)
        w_arr = np.zeros((P, C), np.float32)
        oldoff = np.concatenate([[0], np.cumsum(sk)])
        for j in range(NR):
            wk = int(sk[j])
            if wk == 0:
                continue
            src1[:, off[j]:off[j] + wk] = \
                in_maps[k]["src1"][:, oldoff[j]:oldoff[j] + wk]
            src2[:, off[j]:off[j] + wk] = \
                in_maps[k]["src2"][:, oldoff[j]:oldoff[j] + wk]
            w_arr[:, off[j]:off[j] + wk] = \
                in_maps[k]["w"][:, oldoff[j]:oldoff[j] + wk]
        in_maps[k]["src1"] = src1
        in_maps[k]["src2"] = src2
        in_maps[k]["w"] = w_arr

    common_sections = []
    j = 0
    while j < NR:
        wdt = int(roww[j])
        j2 = j
        while j2 < NR and int(roww[j2]) == wdt:
            j2 += 1
        if wdt > 0:
            common_sections.append((j, j2, wdt, int(off[j])))
        j = j2

    nc = _build_nc(dims, common_sections)
    res = bass_utils.run_bass_kernel_spmd(
        nc, in_maps, core_ids=list(range(NCORES)), trace=_trace)

    out = np.empty(N, np.float32)
    for k in range(NCORES):
        loss_k = res.results[k]["loss"]          # [P, NR]
        D = meta[k]["D"]
        vals = loss_k.T.reshape(-1)              # dealt rank order (j*P+p)
        real = D < NL
        out[k * NL + D[real]] = vals[real]
    if _trace:
        kernel._last_results = res
    return out
